# revision 8
# baseline (speedup 1.0000x reference)
"""Trainium2 Bass kernel for nn_BitwiseLinear: y = x @ tanh(W).T

Full problem: x [32768, 8192] f32, W [256, 8192] f32 -> y [32768, 256] f32.

Data-parallel over 8 NeuronCores: core c computes
    y[c*4096:(c+1)*4096, :] = x_shard @ tanh(W).T
with tanh(W) computed host-side (host prep is not in HW exec time) and
replicated to every core as fp16.

Precision: x is sent as fp8 e3m4 (scaled by 2, clipped to +-15.5) — 4
mantissa bits give ~1.15% end-to-end relative error (gate is 2e-2) and
halve the x DMA traffic (32 MB/core). The PE runs mixed fp16(stationary)
x fp8e3(moving) matmuls at the same 1 col/cycle rate as fp16 (measured
215.8 ns per 512-col matmul), so the kernel is purely PE-bound.

Single weight-stationary phase: the full tanh(W) (4 MB fp16, 32 KB per
partition) stays resident in SBUF; each 512-token chunk accumulates its
full 8192-deep contraction in PSUM (fp32) across 128 matmuls, then one
DVE copy drains it to fp16 and a DMA stores it.

Startup is supply-critical: chunk 0 consumes one x block (64 KB) plus
one w block (64 KB) every 432 ns (~296 GB/s), right at the per-core DMA
rate. Both streams are delivered as matched-block sub-DMAs (w on the ACT
HWDGE queue, x on the SP queue) so descriptor generation runs in
parallel and the PE can consume block groups as they land (subtile
dependency granularity = one sub-DMA).

Device layout (all prepared host-side, so every DMA is contiguous):
  x  -> fp8e3, shard transposed to [tc, p, blk, tl]  (tc = token chunk
        of 512, blk*128+p = contraction index i, tl = token in chunk)
  w  -> fp16 tanh(W).T as [p, blk, o]
  out <- fp16 [256, 4096] = (2*x_shard @ tanh(W).T).T; host divides by 2
"""

import numpy as np
import ml_dtypes

TOKENS = 32768
IN_DIM = 8192
OUT_DIM = 256
N_CORES = 8
TPC = TOKENS // N_CORES        # 4096 tokens per core
TCHUNK = 512                   # tokens per PSUM tile (matmul free dim)
NTC = TPC // TCHUNK            # 8 token chunks per core
P = 128
NBLK = IN_DIM // P             # 64 contraction blocks
NOT = OUT_DIM // P             # 2 output-row tiles
XSCALE = 2.0                   # x pre-scale before e3m4 (max |2x| ~ 12 < 15.5)
NWARM = 26                     # PE warm-up matmuls (HAM clock-gate release)

_NC_CACHE = {}


def _build_nc():
    import concourse.mybir as mybir
    import concourse.tile as tile
    from concourse import bacc

    fp16 = mybir.dt.float16
    fp8 = mybir.dt.float8e3
    f32 = mybir.dt.float32

    nc = bacc.Bacc(
        "TRN2",
        target_bir_lowering=False,
        debug=False,
        num_devices=N_CORES,
        # No SWDGE DMAs in this kernel (all HWDGE via sync/scalar) — reclaim
        # the SBUF descriptor-ring scratch.
        dynamic_dma_scratch_size=2048,
    )
    X = nc.dram_tensor("x", [NTC, P, NBLK, TCHUNK], fp8, kind="ExternalInput").ap()
    W = nc.dram_tensor("w", [P, NBLK, OUT_DIM], fp16, kind="ExternalInput").ap()
    OUT = nc.dram_tensor("out", [OUT_DIM, TPC], fp16, kind="ExternalOutput").ap()

    with tile.TileContext(nc) as tc:
        with (
            tc.tile_pool(name="wtanh", bufs=1) as wt_pool,
            tc.tile_pool(name="xp", bufs=4) as xpool,
            tc.tile_pool(name="yp", bufs=4) as ypool,
            tc.tile_pool(name="ps", bufs=6, space="PSUM") as pspool,
            tc.tile_pool(name="warm", bufs=1, space="PSUM") as warm_pool,
        ):
            wt = wt_pool.tile([P, NBLK, OUT_DIM], fp16, name="wt", tag="wa")

            # PE warm-up: the HAM clock gate keeps the PE at 1.2 GHz until
            # it has been busy ~3.4 us. Run throwaway matmuls on zeroed
            # scratch during the DMA-start dead window so the real stream
            # begins (mostly) at 2.4 GHz.
            scr = ypool.tile([P, P], fp16, name="warm_scr", tag="warm_scr")
            scr_ps = warm_pool.tile([P, P], f32, name="warm_ps", tag="warm_ps")
            nc.vector.memset(scr[:], 0.0)
            for _ in range(NWARM):
                nc.tensor.matmul(
                    scr_ps[:, :], lhsT=scr[:, :], rhs=scr[:, :],
                    start=True, stop=True,
                )

            # Startup ladder: matched-block (x, w) sub-DMA pairs, x on the
            # SP queue and w on the ACT queue (the SDMA engines round-robin
            # between the two queues at ~50/50, and the two streams are the
            # same size, so this balances). Small subs at the head for a
            # fast first matmul, 8 blocks (512 KB per DMA) after.
            x0 = xpool.tile([P, NBLK, TCHUNK], fp8, name="xt0", tag="xt")
            subs = [(0, 2), (2, 2), (4, 4)] + [(j, 8) for j in range(8, NBLK, 8)]
            for j, n in subs:
                nc.sync.dma_start(
                    out=x0[:, j : j + n, :], in_=X[0, :, j : j + n, :]
                )
                nc.scalar.dma_start(
                    out=wt[:, j : j + n, :], in_=W[:, j : j + n, :]
                )

            xtiles = {0: x0}
            # x chunk 1 in quarters, chunk 2 in halves (subtile deps let
            # their matmuls start as soon as each piece lands while the w
            # stream is still finishing); chunks 3+ as single 4 MB DMAs.
            x1 = xpool.tile([P, NBLK, TCHUNK], fp8, name="xt1", tag="xt")
            q = NBLK // 4
            for k in range(4):
                nc.sync.dma_start(
                    out=x1[:, k * q : (k + 1) * q, :],
                    in_=X[1, :, k * q : (k + 1) * q, :],
                )
            xtiles[1] = x1
            x2 = xpool.tile([P, NBLK, TCHUNK], fp8, name="xt2", tag="xt")
            hf = NBLK // 2
            nc.sync.dma_start(out=x2[:, :hf, :], in_=X[2, :, :hf, :])
            nc.sync.dma_start(out=x2[:, hf:, :], in_=X[2, :, hf:, :])
            xtiles[2] = x2

            for t in range(NTC):
                if t not in xtiles:
                    xt = xpool.tile([P, NBLK, TCHUNK], fp8, name=f"xt{t}", tag="xt")
                    nc.sync.dma_start(out=xt[:], in_=X[t, :, :, :])
                    xtiles[t] = xt
                xt = xtiles.pop(t)
                # The very last chunk accumulates into two half-width column
                # groups of its PSUM tiles so half the final drain (DVE copy
                # + store) overlaps the closing matmuls instead of
                # serializing after them.
                last = t == NTC - 1
                NSPL = 2 if last else 1
                NF = TCHUNK // NSPL
                psums = [
                    pspool.tile([P, TCHUNK], f32, name=f"ps_{t}_{o}", tag="ps")
                    for o in range(NOT)
                ]
                for hh in range(NSPL):
                    hsl = slice(hh * NF, (hh + 1) * NF)
                    for bl in range(NBLK):
                        for o in range(NOT):
                            nc.tensor.matmul(
                                psums[o][:, hsl],
                                lhsT=wt[:, bl, o * P : (o + 1) * P],
                                rhs=xt[:, bl, hsl],
                                start=(bl == 0),
                                stop=(bl == NBLK - 1),
                            )
                    if last:
                        # h-outer: half 0's copy+store run while half 1's
                        # matmuls are still streaming.
                        for o in range(NOT):
                            osl = slice(t * TCHUNK + hh * NF,
                                        t * TCHUNK + (hh + 1) * NF)
                            ysb = ypool.tile(
                                [P, NF], fp16, name=f"ysb{t}_{o}_{hh}", tag="ysb"
                            )
                            nc.vector.tensor_copy(ysb[:], psums[o][:, hsl])
                            # Final stores: o=0 on the (now idle) SP queue so
                            # the last descriptor-gens run in parallel.
                            eng = nc.sync if o == 0 else nc.scalar
                            eng.dma_start(
                                out=OUT[o * P : (o + 1) * P, osl], in_=ysb[:]
                            )
                if not last:
                    tsl = slice(t * TCHUNK, (t + 1) * TCHUNK)
                    for o in range(NOT):
                        ysb = ypool.tile(
                            [P, TCHUNK], fp16, name=f"ysb{t}_{o}", tag="ysb"
                        )
                        nc.vector.tensor_copy(ysb[:], psums[o][:, :])
                        # ACT HWDGE queue: w DMAs are long done; don't
                        # serialize behind x loads on the SP queue.
                        nc.scalar.dma_start(
                            out=OUT[o * P : (o + 1) * P, tsl], in_=ysb[:]
                        )
    nc.compile()
    return nc


def _get_nc():
    if "nc" not in _NC_CACHE:
        _NC_CACHE["nc"] = _build_nc()
    return _NC_CACHE["nc"]


def _prep_inputs(x, weight):
    """Host-side tanh + shard + layout. Returns in_maps for the 8 cores."""
    w16 = np.ascontiguousarray(
        np.tanh(weight.astype(np.float32)).T.astype(np.float16)  # [i, o]
        .reshape(NBLK, P, OUT_DIM)           # [blk, p, o]
        .transpose(1, 0, 2)                  # [p, blk, o]
    )
    in_maps = []
    for c in range(N_CORES):
        xc = x[c * TPC : (c + 1) * TPC]      # [4096, 8192] f32
        x8 = np.clip(xc * XSCALE, -15.5, 15.5).astype(ml_dtypes.float8_e3m4)
        xl = np.ascontiguousarray(
            x8.reshape(NTC, TCHUNK, NBLK, P)  # [tc, tl, blk, p]
            .transpose(0, 3, 2, 1)            # [tc, p, blk, tl]
        )
        in_maps.append({"x": xl, "w": w16})
    return in_maps


def run(x, weight, trace=False):
    """Run on hardware; returns (y, BassKernelResults)."""
    from concourse.bass_utils import run_bass_kernel_spmd

    nc = _get_nc()
    in_maps = _prep_inputs(x, weight)
    res = run_bass_kernel_spmd(
        nc, in_maps, core_ids=list(range(N_CORES)), trace=trace
    )
    y = np.concatenate(
        [res.results[c]["out"].astype(np.float32).T for c in range(N_CORES)],
        axis=0,
    ) * (1.0 / XSCALE)
    return y, res


def kernel(x, weight):
    y, _ = run(np.asarray(x), np.asarray(weight), trace=False)
    return y


# revision 9
# speedup vs baseline: 1.0191x; 1.0191x over previous
"""Trainium2 Bass kernel for nn_BitwiseLinear: y = x @ tanh(W).T

Full problem: x [32768, 8192] f32, W [256, 8192] f32 -> y [32768, 256] f32.

Data-parallel over 8 NeuronCores: core c computes
    y[c*4096:(c+1)*4096, :] = x_shard @ tanh(W).T
with tanh(W) computed host-side (host prep is not in HW exec time) and
replicated to every core as fp16.

Precision: x is sent as fp8 e3m4 (scaled by 2, clipped to +-15.5) — 4
mantissa bits give ~1.15% end-to-end relative error (gate is 2e-2) and
halve the x DMA traffic (32 MB/core). The PE runs mixed fp16(stationary)
x fp8e3(moving) matmuls at the same 1 col/cycle rate as fp16 (measured
215.8 ns per 512-col matmul), so the kernel is purely PE-bound.

Single weight-stationary phase: the full tanh(W) (4 MB fp16, 32 KB per
partition) stays resident in SBUF; each 512-token chunk accumulates its
full 8192-deep contraction in PSUM (fp32) across 128 matmuls, then one
DVE copy drains it to fp16 and a DMA stores it.

Startup is supply-critical: chunk 0 consumes one x block (64 KB) plus
one w block (64 KB) every 432 ns (~296 GB/s), right at the per-core DMA
rate. Both streams are delivered as matched-block sub-DMAs (w on the ACT
HWDGE queue, x on the SP queue) so descriptor generation runs in
parallel and the PE can consume block groups as they land (subtile
dependency granularity = one sub-DMA).

Device layout (all prepared host-side, so every DMA is contiguous):
  x  -> fp8e3, shard transposed to [tc, p, blk, tl]  (tc = token chunk
        of 512, blk*128+p = contraction index i, tl = token in chunk)
  w  -> fp16 tanh(W).T as [p, blk, o]
  out <- fp16 [256, 4096] = (2*x_shard @ tanh(W).T).T; host divides by 2
"""

import numpy as np
import ml_dtypes

TOKENS = 32768
IN_DIM = 8192
OUT_DIM = 256
N_CORES = 8
TPC = TOKENS // N_CORES        # 4096 tokens per core
TCHUNK = 512                   # tokens per PSUM tile (matmul free dim)
NTC = TPC // TCHUNK            # 8 token chunks per core
P = 128
NBLK = IN_DIM // P             # 64 contraction blocks
NOT = OUT_DIM // P             # 2 output-row tiles
XSCALE = 2.0
WSCALE = 512.0                   # x pre-scale before e3m4 (max |2x| ~ 12 < 15.5)
NWARM = 26                     # PE warm-up matmuls (HAM clock-gate release)

_NC_CACHE = {}


def _build_nc():
    import concourse.mybir as mybir
    import concourse.tile as tile
    from concourse import bacc

    fp16 = mybir.dt.float16
    fp8 = mybir.dt.float8e3
    f32 = mybir.dt.float32

    nc = bacc.Bacc(
        "TRN2",
        target_bir_lowering=False,
        debug=False,
        num_devices=N_CORES,
        # No SWDGE DMAs in this kernel (all HWDGE via sync/scalar) — reclaim
        # the SBUF descriptor-ring scratch.
        dynamic_dma_scratch_size=2048,
    )
    X = nc.dram_tensor("x", [NTC, P, NBLK, TCHUNK], fp8, kind="ExternalInput").ap()
    W = nc.dram_tensor("w", [P, NBLK, OUT_DIM], fp8, kind="ExternalInput").ap()
    OUT = nc.dram_tensor("out", [OUT_DIM, TPC], fp16, kind="ExternalOutput").ap()

    with tile.TileContext(nc) as tc:
        with (
            tc.tile_pool(name="wtanh", bufs=1) as wt_pool,
            tc.tile_pool(name="xp", bufs=4) as xpool,
            tc.tile_pool(name="yp", bufs=4) as ypool,
            tc.tile_pool(name="ps", bufs=6, space="PSUM") as pspool,
            tc.tile_pool(name="warm", bufs=1, space="PSUM") as warm_pool,
        ):
            wt = wt_pool.tile([P, NBLK, OUT_DIM], fp8, name="wt", tag="wa")

            # PE warm-up: the HAM clock gate keeps the PE at 1.2 GHz until
            # it has been busy ~3.4 us. Run throwaway matmuls on zeroed
            # scratch during the DMA-start dead window so the real stream
            # begins (mostly) at 2.4 GHz.
            scr = ypool.tile([P, P], fp16, name="warm_scr", tag="warm_scr")
            scr_ps = warm_pool.tile([P, P], f32, name="warm_ps", tag="warm_ps")
            nc.vector.memset(scr[:], 0.0)
            for _ in range(NWARM):
                nc.tensor.matmul(
                    scr_ps[:, :], lhsT=scr[:, :], rhs=scr[:, :],
                    start=True, stop=True,
                )

            # Startup ladder: matched-block (x, w) sub-DMA pairs, x on the
            # SP queue and w on the ACT queue (the SDMA engines round-robin
            # between the two queues at ~50/50, and the two streams are the
            # same size, so this balances). Small subs at the head for a
            # fast first matmul, 8 blocks (512 KB per DMA) after.
            x0 = xpool.tile([P, NBLK, TCHUNK], fp8, name="xt0", tag="xt")
            subs = [(0, 2), (2, 2), (4, 4)] + [(j, 8) for j in range(8, NBLK, 8)]
            for j, n in subs:
                nc.sync.dma_start(
                    out=x0[:, j : j + n, :], in_=X[0, :, j : j + n, :]
                )
                nc.scalar.dma_start(
                    out=wt[:, j : j + n, :], in_=W[:, j : j + n, :]
                )

            xtiles = {0: x0}
            # x chunk 1 in quarters, chunk 2 in halves (subtile deps let
            # their matmuls start as soon as each piece lands while the w
            # stream is still finishing); chunks 3+ as single 4 MB DMAs.
            x1 = xpool.tile([P, NBLK, TCHUNK], fp8, name="xt1", tag="xt")
            q = NBLK // 4
            for k in range(4):
                nc.sync.dma_start(
                    out=x1[:, k * q : (k + 1) * q, :],
                    in_=X[1, :, k * q : (k + 1) * q, :],
                )
            xtiles[1] = x1
            x2 = xpool.tile([P, NBLK, TCHUNK], fp8, name="xt2", tag="xt")
            hf = NBLK // 2
            nc.sync.dma_start(out=x2[:, :hf, :], in_=X[2, :, :hf, :])
            nc.sync.dma_start(out=x2[:, hf:, :], in_=X[2, :, hf:, :])
            xtiles[2] = x2

            for t in range(NTC):
                if t not in xtiles:
                    xt = xpool.tile([P, NBLK, TCHUNK], fp8, name=f"xt{t}", tag="xt")
                    nc.sync.dma_start(out=xt[:], in_=X[t, :, :, :])
                    xtiles[t] = xt
                xt = xtiles.pop(t)
                # The very last chunk accumulates into two half-width column
                # groups of its PSUM tiles so half the final drain (DVE copy
                # + store) overlaps the closing matmuls instead of
                # serializing after them.
                last = t == NTC - 1
                NSPL = 2 if last else 1
                NF = TCHUNK // NSPL
                psums = [
                    pspool.tile([P, TCHUNK], f32, name=f"ps_{t}_{o}", tag="ps")
                    for o in range(NOT)
                ]
                for hh in range(NSPL):
                    hsl = slice(hh * NF, (hh + 1) * NF)
                    for bl in range(NBLK):
                        for o in range(NOT):
                            nc.tensor.matmul(
                                psums[o][:, hsl],
                                lhsT=wt[:, bl, o * P : (o + 1) * P],
                                rhs=xt[:, bl, hsl],
                                start=(bl == 0),
                                stop=(bl == NBLK - 1),
                            )
                    if last:
                        # h-outer: half 0's copy+store run while half 1's
                        # matmuls are still streaming.
                        for o in range(NOT):
                            osl = slice(t * TCHUNK + hh * NF,
                                        t * TCHUNK + (hh + 1) * NF)
                            ysb = ypool.tile(
                                [P, NF], fp16, name=f"ysb{t}_{o}_{hh}", tag="ysb"
                            )
                            nc.vector.tensor_copy(ysb[:], psums[o][:, hsl])
                            # Final stores: o=0 on the (now idle) SP queue so
                            # the last descriptor-gens run in parallel.
                            eng = nc.sync if o == 0 else nc.scalar
                            eng.dma_start(
                                out=OUT[o * P : (o + 1) * P, osl], in_=ysb[:]
                            )
                if not last:
                    tsl = slice(t * TCHUNK, (t + 1) * TCHUNK)
                    for o in range(NOT):
                        ysb = ypool.tile(
                            [P, TCHUNK], fp16, name=f"ysb{t}_{o}", tag="ysb"
                        )
                        nc.vector.tensor_copy(ysb[:], psums[o][:, :])
                        # ACT HWDGE queue: w DMAs are long done; don't
                        # serialize behind x loads on the SP queue.
                        nc.scalar.dma_start(
                            out=OUT[o * P : (o + 1) * P, tsl], in_=ysb[:]
                        )
    nc.compile()
    return nc


def _get_nc():
    if "nc" not in _NC_CACHE:
        _NC_CACHE["nc"] = _build_nc()
    return _NC_CACHE["nc"]


def _prep_inputs(x, weight):
    """Host-side tanh + shard + layout. Returns in_maps for the 8 cores."""
    w16 = np.ascontiguousarray(
        (np.tanh(weight.astype(np.float32)).T * WSCALE)
        .astype(ml_dtypes.float8_e3m4)       # [i, o]
        .reshape(NBLK, P, OUT_DIM)           # [blk, p, o]
        .transpose(1, 0, 2)                  # [p, blk, o]
    )
    in_maps = []
    for c in range(N_CORES):
        xc = x[c * TPC : (c + 1) * TPC]      # [4096, 8192] f32
        x8 = np.clip(xc * XSCALE, -15.5, 15.5).astype(ml_dtypes.float8_e3m4)
        xl = np.ascontiguousarray(
            x8.reshape(NTC, TCHUNK, NBLK, P)  # [tc, tl, blk, p]
            .transpose(0, 3, 2, 1)            # [tc, p, blk, tl]
        )
        in_maps.append({"x": xl, "w": w16})
    return in_maps


def run(x, weight, trace=False):
    """Run on hardware; returns (y, BassKernelResults)."""
    from concourse.bass_utils import run_bass_kernel_spmd

    nc = _get_nc()
    in_maps = _prep_inputs(x, weight)
    res = run_bass_kernel_spmd(
        nc, in_maps, core_ids=list(range(N_CORES)), trace=trace
    )
    y = np.concatenate(
        [res.results[c]["out"].astype(np.float32).T for c in range(N_CORES)],
        axis=0,
    ) * (1.0 / (XSCALE * WSCALE))
    return y, res


def kernel(x, weight):
    y, _ = run(np.asarray(x), np.asarray(weight), trace=False)
    return y
